# revision 1
# baseline (speedup 1.0000x reference)
"""GCNConvNet Trainium2 kernel (8 NeuronCores, Bass/Tile).

Dst-sharded graph parallelism, 8 aggregation rounds (A(HW) == (AH)W lets every
round aggregate 64-feature rows):
  - Node features live in an HBM table of bf16 rows padded to 256B (gather
    granule).  Each core owns 12544 destination rows.
  - Per round, each core gathers its edges' source rows with dma_gather
    (int16 indices -> 4 address banks of 25088 rows; 4 SWDGE queues round-
    robined, <=8192 idx/call), then segment-sums them into 32-dst PSUM
    windows with TensorE matmuls against host-built one-hot scatter blocks
    (symmetric-norm coefficients folded into the one-hot values).
  - Dense layer transform + bias/activation runs on the aggregated slice;
    the updated slice is transposed (TensorE) and AllGathered into the
    ping-pong feature tables for the next round.
The block schedule is shared by all cores (single NEFF); per-core differences
live entirely in the input tensors (indices, scatter blocks, x shard).
"""

import sys

sys.path.insert(0, "/opt/trn_rl_repo")

import numpy as np
import ml_dtypes

import concourse.bacc as bacc
import concourse.mybir as mybir
import concourse.tile as tile
from concourse.bass_utils import run_bass_kernel_spmd
from concourse.masks import make_identity

P = 128
HID = 64
FW = 128          # table row width (bf16) = 256B gather granule; cols 64: pad
AFT = mybir.ActivationFunctionType

REAL_CFG = dict(
    N=100000,
    NCORES=8,
    DPC=12544,    # dst rows per core (divisible by 128 and 32)
    W_DST=32,     # dsts per PSUM window
    NROUNDS=8,
    BANK=25088,   # int16-reachable table rows per gather bank
)


def _cfg_derived(cfg):
    c = dict(cfg)
    c["NW"] = c["DPC"] // c["W_DST"]
    c["NTOT"] = c["DPC"] * c["NCORES"]       # table rows (= padded node count)
    c["NBANK"] = -(-c["NTOT"] // c["BANK"])
    c["J"] = c["DPC"] // P
    c["WPG"] = 512 // c["W_DST"]             # windows per 512-col PSUM group
    c["NG"] = -(-c["NW"] // c["WPG"])
    return c


# ---------------------------------------------------------------- host side --
def preprocess(edge_index, cfg):
    """Slot/scatter schedule shared by all cores + per-core idx / S tensors.

    Slot order: (psum-group g, bank b, window w, dst, edge).  Within each
    (g,b): per-window slot counts are equalized across cores (max), then the
    (g,b) range is padded to a multiple of 128.  Slot s of a gather call maps
    to m-tile position (lane s%128, col s//128).

    Matmuls: one per (col, window-pair); rhs S[:, mm, 0:64] covers psum cols
    [wbase*32, wbase*32+64).
    """
    N, NCORES, DPC, W = cfg["N"], cfg["NCORES"], cfg["DPC"], cfg["W_DST"]
    NW, WPG, NG, BANK = cfg["NW"], cfg["WPG"], cfg["NG"], cfg["BANK"]

    src = np.concatenate([edge_index[0], np.arange(N)]).astype(np.int64)
    dst = np.concatenate([edge_index[1], np.arange(N)]).astype(np.int64)
    deg = np.bincount(dst, minlength=N).astype(np.float64)
    dinv = deg ** -0.5
    norm = (dinv[src] * dinv[dst]).astype(np.float32)

    core = dst // DPC
    win = (dst % DPC) // W          # window within core [0, NW)
    grp = win // WPG                # psum group [0, NG)
    bank = src // BANK              # gather bank [0, NBANK)
    dloc = dst % W

    # per-(core, g, b, w) counts -> equalized across cores
    NB_ = cfg["NBANK"]
    key_w = ((core * NG + grp) * NB_ + bank) * NW + win   # coarse unique key
    cnt = np.bincount(key_w, minlength=NCORES * NG * NB_ * NW).reshape(
        NCORES, NG, NB_, NW
    )
    cnt_eq = cnt.max(axis=0)                               # [NG, NB, NW]

    # slot base for each (g, b, w) in the shared schedule
    flat = cnt_eq.reshape(-1)
    base_w = np.concatenate([[0], np.cumsum(flat)])        # uneq-padded bases
    # pad each (g,b) range to 128
    gb_sizes = cnt_eq.sum(axis=2).reshape(-1)              # [NG*NB]
    gb_pad = (-gb_sizes) % P
    gb_sizes_p = gb_sizes + gb_pad
    gb_base = np.concatenate([[0], np.cumsum(gb_sizes_p)])
    NSLOT = int(gb_base[-1])

    # base of window w within its (g,b) block
    w_off = np.zeros_like(cnt_eq)
    w_off[:, :, 1:] = np.cumsum(cnt_eq, axis=2)[:, :, :-1]
    w_base = gb_base[:-1].reshape(NG, NB_) [:, :, None] + w_off  # [NG,NB,NW]

    # edge -> slot
    order = np.lexsort((dst, bank, grp, core))
    srcs, _dsts, norms = src[order], dst[order], norm[order]
    cores, grps, banks, wins, dlocs = (
        core[order], grp[order], bank[order], win[order], dloc[order]
    )
    key = ((cores * NG + grps) * NB_ + banks) * NW + wins
    starts = np.zeros(NCORES * NG * NB_ * NW + 1, np.int64)
    np.add.at(starts, key + 1, 1)
    starts = np.cumsum(starts)
    rank = np.arange(len(key)) - starts[key]               # pos within group
    slot = w_base[grps, banks, wins] + rank

    lane = slot % P
    col = slot // P
    NCOL = NSLOT // P

    # idx values: row within bank (int16); pad slots point at row 0 (S=0)
    idx = np.zeros((NCORES, P, NCOL), np.int16)
    idx[cores, lane, col] = (srcs % BANK).astype(np.int16)

    # ---- matmul schedule (shared) ----
    # per column: windows present = from cnt_eq geometry (not data!)
    # col range of window w: [w_base, w_base+cnt_eq) -> cols touched
    mm_col, mm_wb, mm_g = [], [], []
    col2mm0 = np.zeros(NCOL + 1, np.int64)
    win_first_mm = {}
    win_last_mm = {}
    for g in range(NG):
        for b in range(NB_):
            for w in range(WPG * g, min(WPG * (g + 1), NW)):
                c0 = int(w_base[g, b, w - 0] // 1)
                n = int(cnt_eq[g, b, w])
                if n == 0:
                    continue
                c_first, c_last = c0 // P, (c0 + n - 1) // P
                for c in range(c_first, c_last + 1):
                    mm_col.append(c)
                    mm_wb.append(w)
                    mm_g.append(g)
    # first/last pre-merge entry per window
    n_pre = len(mm_col)
    first_pre, last_pre = {}, {}
    for i, w in enumerate(mm_wb):
        if w not in first_pre:
            first_pre[w] = i
        last_pre[w] = i

    # merge adjacent-window same-col entries into N=64 pairs when their
    # start/stop parity matches (PSUM start zeroes written cols only).
    merged = []  # (col, wbase, g, [windows], n32)  n32: rhs width in windows
    i = 0
    while i < n_pre:
        c, w, g = mm_col[i], mm_wb[i], mm_g[i]
        can = (
            i + 1 < n_pre
            and mm_col[i + 1] == c
            and mm_g[i + 1] == g
            and mm_wb[i + 1] == w + 1
            and (first_pre[w] == i) == (first_pre[w + 1] == i + 1)
            and (last_pre[w] == i) == (last_pre[w + 1] == i + 1)
        )
        if can:
            merged.append((c, w, g, [w, w + 1]))
            i += 2
        else:
            merged.append((c, w, g, [w]))
            i += 1
    NMM = len(merged)

    first_of_w, last_of_w = {}, {}
    for m, (c, wb, g, ws) in enumerate(merged):
        for w in ws:
            if w not in first_of_w:
                first_of_w[w] = m
            last_of_w[w] = m
    mm_start = np.zeros(NMM, bool)
    mm_stop = np.zeros(NMM, bool)
    mm_n = np.zeros(NMM, np.int64)
    for m, (c, wb, g, ws) in enumerate(merged):
        mm_start[m] = first_of_w[ws[0]] == m
        mm_stop[m] = last_of_w[ws[-1]] == m
        mm_n[m] = len(ws) * W

    # S blocks [P, NMM, 64]
    smat = np.zeros((NCORES, P, NMM, 64), np.float32)
    mm_lookup = {}
    for m, (c, wb, g, ws) in enumerate(merged):
        for w in ws:
            mm_lookup[(c, w)] = (m, wb)
    pair_keys = col * (NW + 1) + wins
    uniq, inv = np.unique(pair_keys, return_inverse=True)
    mm_u = np.empty(len(uniq), np.int64)
    off_u = np.empty(len(uniq), np.int64)
    for i2, pk in enumerate(uniq):
        c2, w2 = int(pk // (NW + 1)), int(pk % (NW + 1))
        m, wb = mm_lookup[(c2, w2)]
        mm_u[i2] = m
        off_u[i2] = (w2 - wb) * W
    mm_of_edge = mm_u[inv]
    off_of_edge = off_u[inv]
    smat[cores, lane, mm_of_edge, off_of_edge + dlocs] = norms

    groups = []
    for g in range(NG):
        b0 = int(gb_base[g * NB_] // P)
        b1 = int(gb_base[(g + 1) * NB_] // P)
        # per-bank col ranges + idx counts
        bank_cols = [
            (
                int(gb_base[g * NB_ + b] // P),
                int(gb_base[g * NB_ + b + 1] // P),
            )
            for b in range(NB_)
        ]
        w0, w1 = WPG * g, min(WPG * (g + 1), NW)
        mm_range = [m for m, mm in enumerate(merged) if mm[2] == g]
        groups.append(
            dict(
                cols=(b0, b1),
                bank_cols=bank_cols,
                ncols_psum=(w1 - w0) * W,
                mm0=min(mm_range),
                mm1=max(mm_range) + 1,
            )
        )
    # mm list entries per group must be contiguous
    for g, gr in enumerate(groups):
        for m in range(gr["mm0"], gr["mm1"]):
            assert merged[m][2] == g

    sched = dict(
        merged=merged,
        mm_start=mm_start,
        mm_stop=mm_stop,
        mm_n=mm_n,
        groups=groups,
        NMM=NMM,
        NCOL=NCOL,
        NSLOT=NSLOT,
    )
    return dict(
        idx=idx,
        smat=smat.astype(ml_dtypes.bfloat16),
        sched=sched,
    )


# -------------------------------------------------------------- device side --
def build_program(cfg, sched):
    NCORES, DPC, W = cfg["NCORES"], cfg["DPC"], cfg["W_DST"]
    NW, NTOT, J, WPG, NG, BANK = (
        cfg["NW"], cfg["NTOT"], cfg["J"], cfg["WPG"], cfg["NG"], cfg["BANK"]
    )
    NROUNDS = cfg["NROUNDS"]
    NHID = max(NROUNDS - 2, 0)
    NB_ = cfg["NBANK"]
    f32, bf16, i16 = mybir.dt.float32, mybir.dt.bfloat16, mybir.dt.int16
    merged, mm_start, mm_stop, mm_n, groups, NMM, NCOL = (
        sched["merged"], sched["mm_start"], sched["mm_stop"], sched["mm_n"],
        sched["groups"], sched["NMM"], sched["NCOL"],
    )

    nc = bacc.Bacc(
        "TRN2", target_bir_lowering=False, debug=False,
        num_devices=NCORES, num_swdge_queues=4,
    )

    idx_t = nc.dram_tensor("idx", [P, NCOL * 8], i16, kind="ExternalInput")
    s_t = nc.dram_tensor("smat", [P, NMM, 64], bf16, kind="ExternalInput")
    x_t = nc.dram_tensor("xsh", [DPC, 3], f32, kind="ExternalInput")
    win_t = nc.dram_tensor("w_in", [3, HID], f32, kind="ExternalInput")
    bin_t = nc.dram_tensor("b_in", [HID, 1], f32, kind="ExternalInput")
    whid_t = nc.dram_tensor("w_hid", [max(NHID, 1), HID, HID], bf16, kind="ExternalInput")
    bhid_t = nc.dram_tensor("b_hid", [max(NHID, 1), HID, 1], f32, kind="ExternalInput")
    wout_t = nc.dram_tensor("w_out", [HID, 6], bf16, kind="ExternalInput")
    bout_t = nc.dram_tensor("b_out", [6, 1], f32, kind="ExternalInput")
    y_t = nc.dram_tensor("y", [DPC, 6], f32, kind="ExternalOutput")

    tables = [
        nc.dram_tensor(f"table{i}", [NTOT, FW], bf16, addr_space="Shared")
        for i in range(2)
    ]
    hsl = [nc.dram_tensor(f"hslice{i}", [DPC, FW], bf16) for i in range(2)]
    rg = [list(range(NCORES))]

    cmax = max(gr["cols"][1] - gr["cols"][0] for gr in groups)
    bmax = max(
        c1 - c0 for gr in groups for (c0, c1) in gr["bank_cols"]
    )
    smax = max((gr["mm1"] - gr["mm0"] + 1) // 2 + 1 for gr in groups)
    # col -> bank lookup
    col_bank = np.zeros(NCOL, np.int64)
    for gr in groups:
        for b, (c0, c1) in enumerate(gr["bank_cols"]):
            col_bank[c0:c1] = b

    with tile.TileContext(nc, num_cores=NCORES) as tc:
        with (
            tc.tile_pool(name="const", bufs=1) as cpool,
            tc.tile_pool(name="mp", bufs=5) as mpool,
            tc.tile_pool(name="sp", bufs=3) as spool,
            tc.tile_pool(name="ip", bufs=2) as ipool,
            tc.tile_pool(name="atp", bufs=1) as atpool,
            tc.tile_pool(name="trp", bufs=1) as trpool,
            tc.tile_pool(name="rhp", bufs=2) as rhpool,
            tc.tile_pool(name="ps_sc", bufs=2, space="PSUM") as ps_sc,
            tc.tile_pool(name="ps_tr", bufs=2, space="PSUM") as ps_tr,
            tc.tile_pool(name="ps_tp", bufs=2, space="PSUM") as ps_tp,
        ):
            # ---- constants ----
            ident_f = cpool.tile([P, P], f32, tag="idf")
            make_identity(nc, ident_f[:])
            ident_b = cpool.tile([P, P], bf16, tag="idb")
            make_identity(nc, ident_b[:])
            w_in_sb = cpool.tile([3, HID], f32, tag="wi")
            nc.sync.dma_start(out=w_in_sb[:], in_=win_t[:])
            b_in_sb = cpool.tile([HID, 1], f32, tag="bi")
            nc.sync.dma_start(out=b_in_sb[:], in_=bin_t[:])
            whid_sb = cpool.tile([HID, max(NHID, 1) * HID], bf16, tag="wh")
            bhid_sb = cpool.tile([HID, max(NHID, 1)], f32, tag="bh")
            for l in range(max(NHID, 1)):
                nc.sync.dma_start(
                    out=whid_sb[:, l * HID : (l + 1) * HID], in_=whid_t[l, :, :]
                )
                nc.sync.dma_start(out=bhid_sb[:, l : l + 1], in_=bhid_t[l, :, :])
            wout_sb = cpool.tile([HID, 6], bf16, tag="wo")
            nc.sync.dma_start(out=wout_sb[:], in_=wout_t[:])
            bout_sb = cpool.tile([6, 1], f32, tag="bo")
            nc.sync.dma_start(out=bout_sb[:], in_=bout_t[:])

            # ---- round 0 table: t0 = x @ W_in ----
            htr = trpool.tile([P, J * FW], bf16, tag="htr")
            nc.gpsimd.memset(htr[:], 0.0)
            for j in range(J):
                xc = rhpool.tile([P, 3], f32, tag="xc")
                nc.sync.dma_start(out=xc[:], in_=x_t[j * P : (j + 1) * P, :])
                pxT = ps_tp.tile([3, P], f32, space="PSUM", tag="ptp")
                nc.tensor.transpose(out=pxT[:], in_=xc[:], identity=ident_f[:])
                xT = rhpool.tile([3, P], f32, tag="xT")
                nc.vector.tensor_copy(out=xT[:], in_=pxT[:])
                pt0 = ps_tr.tile([P, HID], f32, space="PSUM", tag="ptr")
                nc.tensor.matmul(
                    out=pt0[:], lhsT=xT[:], rhs=w_in_sb[:], start=True, stop=True
                )
                nc.scalar.activation(
                    out=htr[:, j * FW : j * FW + HID], in_=pt0[:], func=AFT.Copy
                )
            nc.sync.dma_start(
                out=hsl[0].ap().rearrange("(j p) f -> p j f", p=P),
                in_=htr[:].rearrange("p (j f) -> p j f", f=FW),
            )
            nc.gpsimd.collective_compute(
                "AllGather", mybir.AluOpType.bypass, replica_groups=rg,
                ins=[hsl[0][:]], outs=[tables[0][:, :]],
            )

            # ---- rounds ----
            qn = 0
            for r in range(NROUNDS):
                table = tables[r % 2]
                at_sb = atpool.tile([HID, DPC], bf16, tag="at")
                for gi, gr in enumerate(groups):
                    b0, b1 = gr["cols"]
                    idx_sb = ipool.tile([P, cmax * 8], i16, tag="ix")
                    nc.sync.dma_start(
                        out=idx_sb[:, 0 : (b1 - b0) * 8],
                        in_=idx_t[:, b0 * 8 : b1 * 8],
                    )
                    nmm_g = gr["mm1"] - gr["mm0"]
                    mid = gr["mm0"] + (nmm_g + 1) // 2
                    s_lo = spool.tile([P, smax, 64], bf16, tag="s")
                    nc.sync.dma_start(
                        out=s_lo[:, 0 : mid - gr["mm0"], :],
                        in_=s_t[:, gr["mm0"] : mid, :],
                    )
                    s_hi = spool.tile([P, smax, 64], bf16, tag="s")
                    nc.sync.dma_start(
                        out=s_hi[:, 0 : gr["mm1"] - mid, :],
                        in_=s_t[:, mid : gr["mm1"], :],
                    )
                    mtiles = []
                    for b in range(NB_):
                        c0, c1 = gr["bank_cols"][b]
                        if c1 == c0:
                            mtiles.append(None)
                            continue
                        mt = mpool.tile([P, bmax, FW], bf16, tag="m")
                        nidx = (c1 - c0) * P
                        nc.gpsimd.dma_gather(
                            out_ap=mt[:, 0 : c1 - c0, :],
                            in_ap=table[b * BANK : min((b + 1) * BANK, NTOT), :],
                            idxs_ap=idx_sb[:, (c0 - b0) * 8 : (c1 - b0) * 8],
                            num_idxs=nidx,
                            num_idxs_reg=nidx,
                            elem_size=FW,
                            single_packet=False,
                            queue_num=qn % 4,
                        )
                        qn += 1
                        mtiles.append((mt, c0))
                    psum = ps_sc.tile([HID, 512], f32, space="PSUM", tag="psc")
                    for m in range(gr["mm0"], gr["mm1"]):
                        c, wb, g, ws = merged[m]
                        wl = wb - WPG * g
                        nn = int(mm_n[m])
                        bk = col_bank[c]
                        mt, cb = mtiles[bk]
                        s_sb, sbase = (
                            (s_lo, gr["mm0"]) if m < mid else (s_hi, mid)
                        )
                        nc.tensor.matmul(
                            out=psum[:, wl * W : wl * W + nn],
                            lhsT=mt[:, c - cb, 0:HID],
                            rhs=s_sb[:, m - sbase, 0:nn],
                            start=bool(mm_start[m]),
                            stop=bool(mm_stop[m]),
                            skip_group_check=True,
                        )
                    nc.scalar.activation(
                        out=at_sb[:, gi * 512 : gi * 512 + gr["ncols_psum"]],
                        in_=psum[:, 0 : gr["ncols_psum"]],
                        func=AFT.Copy,
                    )

                # ---- transform + transpose (fused per 512-col chunk) ----
                htr2 = trpool.tile([P, J * FW], bf16, tag="htr")
                ytr = None
                if r == NROUNDS - 1:
                    ytr = trpool.tile([P, J * 6], f32, tag="ytr")
                elif r == 0:
                    nc.gpsimd.memset(htr2[:], 0.0)
                nch = -(-DPC // 512)
                for ch in range(nch):
                    sl = slice(ch * 512, min((ch + 1) * 512, DPC))
                    ncol = sl.stop - sl.start
                    if r == NROUNDS - 1:
                        yc = rhpool.tile([6, 512], f32, tag="yc")
                        pt = ps_tr.tile([6, 512], f32, space="PSUM", tag="ptr")
                        nc.tensor.matmul(
                            out=pt[:, 0:ncol], lhsT=wout_sb[:],
                            rhs=at_sb[:, sl], start=True, stop=True,
                        )
                        nc.scalar.activation(
                            out=yc[:, 0:ncol], in_=pt[:, 0:ncol],
                            func=AFT.Sigmoid, bias=bout_sb[:],
                        )
                        for jj in range(ncol // P):
                            j = ch * 4 + jj
                            ptp6 = ps_tp.tile([P, 6], f32, space="PSUM", tag="ptp")
                            nc.tensor.transpose(
                                out=ptp6[:], in_=yc[:, jj * P : (jj + 1) * P],
                                identity=ident_f[0:6, 0:6],
                            )
                            nc.vector.tensor_copy(
                                out=ytr[:, j * 6 : (j + 1) * 6], in_=ptp6[:]
                            )
                        continue
                    hc = rhpool.tile([HID, 512], bf16, tag="hc")
                    if r == 0:
                        nc.scalar.activation(
                            out=hc[:, 0:ncol], in_=at_sb[:, sl], func=AFT.Relu,
                            bias=b_in_sb[:],
                        )
                    else:
                        pt = ps_tr.tile([HID, 512], f32, space="PSUM", tag="ptr")
                        nc.tensor.matmul(
                            out=pt[:, 0:ncol],
                            lhsT=whid_sb[:, (r - 1) * HID : r * HID],
                            rhs=at_sb[:, sl], start=True, stop=True,
                        )
                        nc.scalar.activation(
                            out=hc[:, 0:ncol], in_=pt[:, 0:ncol], func=AFT.Relu,
                            bias=bhid_sb[:, r - 1 : r],
                        )
                    for jj in range(ncol // P):
                        j = ch * 4 + jj
                        ptp = ps_tp.tile([P, HID], bf16, space="PSUM", tag="ptp")
                        nc.tensor.transpose(
                            out=ptp[:], in_=hc[:, jj * P : (jj + 1) * P],
                            identity=ident_b[0:HID, 0:HID],
                        )
                        nc.scalar.activation(
                            out=htr2[:, j * FW : j * FW + HID], in_=ptp[:],
                            func=AFT.Copy,
                        )

                # ---- publish ----
                if r < NROUNDS - 1:
                    dst_h = hsl[(r + 1) % 2]
                    nc.sync.dma_start(
                        out=dst_h.ap().rearrange("(j p) f -> p j f", p=P),
                        in_=htr2[:].rearrange("p (j f) -> p j f", f=FW),
                    )
                    nc.gpsimd.collective_compute(
                        "AllGather", mybir.AluOpType.bypass, replica_groups=rg,
                        ins=[dst_h[:]], outs=[tables[(r + 1) % 2][:, :]],
                    )
                else:
                    nc.sync.dma_start(
                        out=y_t.ap().rearrange("(j p) f -> p j f", p=P),
                        in_=ytr[:].rearrange("p (j f) -> p j f", f=6),
                    )

    nc.compile()
    return nc


# ----------------------------------------------------------------- assembly --
def make_in_maps(inputs, pre, cfg):
    N, NCORES, DPC = cfg["N"], cfg["NCORES"], cfg["DPC"]
    NHID = max(cfg["NROUNDS"] - 2, 0)
    x = np.asarray(inputs["x"], np.float32)
    xpad = np.zeros((NCORES * DPC, 3), np.float32)
    xpad[:N] = x
    w_in = np.asarray(inputs["W_in"], np.float32)
    b_in = np.asarray(inputs["b_in"], np.float32).reshape(HID, 1)
    w_hid = np.asarray(inputs["W_hid"], np.float32)[:NHID]
    b_hid = np.asarray(inputs["b_hid"], np.float32)[:NHID]
    if NHID == 0:
        w_hid = np.zeros((1, HID, HID), np.float32)
        b_hid = np.zeros((1, HID), np.float32)
    w_out = np.asarray(inputs["W_out"], np.float32)
    b_out = np.asarray(inputs["b_out"], np.float32).reshape(6, 1)

    # idx wrapped-16 + replicated across the 8 Q7 cores:
    # partition p holds indices of lane p%16: i.e. for positions pos with
    # pos%16 == p%16, laid at column pos//16.
    idxw = []
    for k in range(NCORES):
        a = pre["idx"][k]               # [P, NCOL] slot layout (lane, col)
        # slot pos = col*128 + lane ; gather wants [16, num/16] wrapped:
        # w16[i%16, i//16] = idx[pos=i]
        ncol = a.shape[1]
        flat = a.T.reshape(-1)          # pos order: col-major -> pos = c*128+p
        w16 = flat.reshape(-1, 16).T    # [16, NSLOT/16]
        idxw.append(np.ascontiguousarray(np.tile(w16, (8, 1))))

    in_maps = []
    for k in range(NCORES):
        in_maps.append(
            {
                "idx": idxw[k],
                "smat": np.ascontiguousarray(pre["smat"][k]),
                "xsh": np.ascontiguousarray(xpad[k * DPC : (k + 1) * DPC]),
                "w_in": w_in,
                "b_in": b_in,
                "w_hid": w_hid.astype(ml_dtypes.bfloat16),
                "b_hid": np.ascontiguousarray(b_hid.reshape(-1, HID, 1)).astype(np.float32),
                "w_out": w_out.astype(ml_dtypes.bfloat16),
                "b_out": b_out,
            }
        )
    return in_maps


def run(inputs, cfg=None, **spmd_kwargs):
    cfg = _cfg_derived(dict(cfg or REAL_CFG))
    edge_index = np.asarray(inputs["edge_index"])
    pre = preprocess(edge_index, cfg)
    nc = build_program(cfg, pre["sched"])
    in_maps = make_in_maps(inputs, pre, cfg)
    res = run_bass_kernel_spmd(
        nc, in_maps, core_ids=list(range(cfg["NCORES"])), **spmd_kwargs
    )
    y = np.concatenate([res.results[k]["y"] for k in range(cfg["NCORES"])])
    return y[: cfg["N"]].astype(np.float32), res


def kernel(**inputs):
    y, _ = run(inputs)
    return y



# revision 2
# speedup vs baseline: 1.1415x; 1.1415x over previous
"""GCNConvNet Trainium2 kernel v3 (8 NeuronCores, Bass/Tile).

Dst-sharded graph parallelism, 8 aggregation rounds (A(HW) == (AH)W).
v2 over the baseline:
  - Scatter one-hot S blocks are built ON-CHIP (DVE iota/is_equal/mult from a
    compact [P, NMM] dstoff/norm encoding) instead of streaming 72MB of
    pre-built S from HBM every round.
  - Self-loop edges are folded into per-tile diagonal matmuls against the
    previous round's resident transposed slice (htr ping-pong) — removes
    ~12.5K gather slots/core/round and zero-initializes PSUM (start=True).
  - The feature table is laid out chunk-major so each 5-group chunk of the
    transformed slice is published + AllGathered as soon as it is ready,
    hiding the collective behind the next groups' gather descgen.
  - 64-dst windows (no pair merging): equalization waste drops.
The gather itself (SWDGE dma_gather, 256B elements, int16 bank indices,
4 queues) is unchanged — Q7 descriptor generation is the critical resource.
"""

import sys

sys.path.insert(0, "/opt/trn_rl_repo")

import numpy as np
import ml_dtypes

import concourse.bacc as bacc
import concourse.mybir as mybir
import concourse.tile as tile
from concourse.bass_utils import run_bass_kernel_spmd
from concourse.masks import make_identity

P = 128
HID = 64
FW = 128          # table row width (bf16) = 256B gather granule; cols 64: pad
AFT = mybir.ActivationFunctionType

REAL_CFG = dict(
    N=100000,
    NCORES=8,
    DPC=12544,    # dst rows per core (divisible by 512)
    W_DST=64,     # dsts per window (= rhs width per scatter matmul)
    NROUNDS=8,
    BANK=25088,   # int16-reachable table rows per gather bank
    CHUNK_ENDS=(6, 12, 18, 24, 25),  # group index ends of collective chunks
)


def _cfg_derived(cfg):
    c = dict(cfg)
    c["NW"] = c["DPC"] // c["W_DST"]
    c["NTOT"] = c["DPC"] * c["NCORES"]
    c["NBANK"] = -(-c["NTOT"] // c["BANK"])
    c["J"] = c["DPC"] // P
    c["WPG"] = 512 // c["W_DST"]
    c["NG"] = -(-c["NW"] // c["WPG"])
    ends = list(c["CHUNK_ENDS"])
    assert ends[-1] == c["NG"] if "NG" in c else True
    c["NCHUNK"] = len(ends)
    bounds = [0] + [min(e * 512, c["DPC"]) for e in ends]
    c["CSZ"] = [bounds[i + 1] - bounds[i] for i in range(len(ends))]
    c["CBASE"] = np.asarray(bounds[:-1], np.int64)
    c["CBOUNDS"] = np.asarray(bounds, np.int64)
    return c


def _table_row(node, cfg):
    """Chunk-major table row for a global node id (nonuniform chunks)."""
    DPC = cfg["DPC"]
    k = node // DPC
    d = node % DPC
    c = np.searchsorted(cfg["CBOUNDS"][1:], d, side="right")
    sz = np.asarray(cfg["CSZ"], np.int64)
    base = np.asarray(cfg["CBASE"], np.int64)
    return base[c] * cfg["NCORES"] + k * sz[c] + (d - base[c])


# ---------------------------------------------------------------- host side --
def preprocess(edge_index, cfg):
    """Slot/matmul schedule shared by all cores + per-core idx/dstoff/snorm.

    Slot order: (psum-group g, bank b, window w, edge).  Per (g,b,w) slot
    counts are equalized across cores (max); each (g,b) range is padded to a
    multiple of 128.  Slot s maps to (lane s%128, col s//128).  One matmul
    per (col, window): rhs = on-chip-built one-hot S [128, 64].
    Self-loops are NOT in the slot list (handled by diag matmuls on-device).
    """
    N, NCORES, DPC, W = cfg["N"], cfg["NCORES"], cfg["DPC"], cfg["W_DST"]
    NW, WPG, NG, BANK, NB_ = cfg["NW"], cfg["WPG"], cfg["NG"], cfg["BANK"], cfg["NBANK"]
    J = cfg["J"]

    src = edge_index[0].astype(np.int64)
    dst = edge_index[1].astype(np.int64)
    deg = np.bincount(dst, minlength=N).astype(np.float64) + 1.0  # + self loop
    dinv = deg ** -0.5
    norm = (dinv[src] * dinv[dst]).astype(np.float32)

    # per-core dinv [P, J] and [64, DPC] row replica (pad nodes: dinv=1)
    dinv_pad = np.ones(cfg["NTOT"], np.float64)
    dinv_pad[:N] = dinv
    dinvp = dinv_pad.reshape(NCORES, J, P).transpose(0, 2, 1)
    dinvcol = np.ascontiguousarray(
        np.broadcast_to(
            dinv_pad.reshape(NCORES, 1, cfg["DPC"]), (NCORES, HID, cfg["DPC"])
        )
    )

    row = _table_row(src, cfg)          # chunk-major table rows of sources
    core = dst // DPC
    win = (dst % DPC) // W
    grp = win // WPG
    bank = row // BANK
    dloc = dst % W

    cnt = np.bincount(
        ((core * NG + grp) * NB_ + bank) * NW + win,
        minlength=NCORES * NG * NB_ * NW,
    ).reshape(NCORES, NG, NB_, NW)
    cnt_eq = cnt.max(axis=0)                               # [NG, NB, NW]

    gb_sizes = cnt_eq.sum(axis=2).reshape(-1)              # [NG*NB]
    gb_sizes_p = gb_sizes + (-gb_sizes) % P
    gb_base = np.concatenate([[0], np.cumsum(gb_sizes_p)])
    NSLOT = int(gb_base[-1])

    w_off = np.zeros_like(cnt_eq)
    w_off[:, :, 1:] = np.cumsum(cnt_eq, axis=2)[:, :, :-1]
    w_base = gb_base[:-1].reshape(NG, NB_)[:, :, None] + w_off  # [NG,NB,NW]

    order = np.lexsort((dst, win, bank, grp, core))
    srcr = row[order]
    cores, grps, banks, wins, dlocs = (
        core[order], grp[order], bank[order], win[order], dloc[order]
    )
    key = ((cores * NG + grps) * NB_ + banks) * NW + wins
    starts = np.zeros(NCORES * NG * NB_ * NW + 1, np.int64)
    np.add.at(starts, key + 1, 1)
    starts = np.cumsum(starts)
    rank = np.arange(len(key)) - starts[key]
    slot = w_base[grps, banks, wins] + rank

    lane = slot % P
    col = slot // P
    NCOL = NSLOT // P

    idx = np.zeros((NCORES, P, NCOL), np.int16)
    idx[cores, lane, col] = (srcr % BANK).astype(np.int16)

    # ---- matmul schedule (shared, per (g,b,w) col entries) ----
    mm_col, mm_w, mm_g = [], [], []
    for g in range(NG):
        for b in range(NB_):
            for w in range(WPG * g, min(WPG * (g + 1), NW)):
                n = int(cnt_eq[g, b, w])
                if n == 0:
                    continue
                c0 = int(w_base[g, b, w])
                for c in range(c0 // P, (c0 + n - 1) // P + 1):
                    mm_col.append(c)
                    mm_w.append(w)
                    mm_g.append(g)
    NMM = len(mm_col)
    last_of_w = {}
    for m, w in enumerate(mm_w):
        last_of_w[w] = m
    mm_stop = np.zeros(NMM, bool)
    for w, m in last_of_w.items():
        mm_stop[m] = True

    # edge -> matmul id
    mm_lookup = {}
    for m in range(NMM):
        mm_lookup[(mm_col[m], mm_w[m])] = m
    pair_keys = col * NW + wins
    uniq, inv = np.unique(pair_keys, return_inverse=True)
    mm_u = np.empty(len(uniq), np.int64)
    for i2, pk in enumerate(uniq):
        mm_u[i2] = mm_lookup[(int(pk // NW), int(pk % NW))]
    mm_of_edge = mm_u[inv]

    dstoff = np.full((NCORES, P, NMM), -1, np.int16)
    dstoff[cores, lane, mm_of_edge] = dlocs.astype(np.int16)

    groups = []
    for g in range(NG):
        bank_cols = [
            (int(gb_base[g * NB_ + b] // P), int(gb_base[g * NB_ + b + 1] // P))
            for b in range(NB_)
        ]
        w0, w1 = WPG * g, min(WPG * (g + 1), NW)
        mm_range = [m for m in range(NMM) if mm_g[m] == g]
        groups.append(
            dict(
                cols=(int(gb_base[g * NB_] // P), int(gb_base[(g + 1) * NB_] // P)),
                bank_cols=bank_cols,
                ncols_psum=(w1 - w0) * W,
                mm0=min(mm_range),
                mm1=max(mm_range) + 1,
            )
        )
    for g, gr in enumerate(groups):
        for m in range(gr["mm0"], gr["mm1"]):
            assert mm_g[m] == g

    sched = dict(
        mm_col=np.asarray(mm_col, np.int64),
        mm_w=np.asarray(mm_w, np.int64),
        mm_stop=mm_stop,
        groups=groups,
        NMM=NMM,
        NCOL=NCOL,
        NSLOT=NSLOT,
    )
    return dict(
        idx=idx,
        dstoff=dstoff,
        dinvp=dinvp.astype(np.float32),
        dinvcol=dinvcol.astype(np.float32),
        sched=sched,
    )


# -------------------------------------------------------------- device side --
def build_program(cfg, sched):
    NCORES, DPC, W = cfg["NCORES"], cfg["DPC"], cfg["W_DST"]
    NW, NTOT, J, WPG, NG, BANK = (
        cfg["NW"], cfg["NTOT"], cfg["J"], cfg["WPG"], cfg["NG"], cfg["BANK"]
    )
    NROUNDS = cfg["NROUNDS"]
    NHID = max(NROUNDS - 2, 0)
    NB_ = cfg["NBANK"]
    NCHUNK, CSZ, CBASE = cfg["NCHUNK"], cfg["CSZ"], cfg["CBASE"]
    CHUNK_ENDS = list(cfg["CHUNK_ENDS"])
    f32, bf16, i16 = mybir.dt.float32, mybir.dt.bfloat16, mybir.dt.int16
    mm_col, mm_w, mm_stop, groups, NMM, NCOL = (
        sched["mm_col"], sched["mm_w"], sched["mm_stop"],
        sched["groups"], sched["NMM"], sched["NCOL"],
    )

    nc = bacc.Bacc(
        "TRN2", target_bir_lowering=False, debug=False,
        num_devices=NCORES, num_swdge_queues=4,
    )

    idx_t = nc.dram_tensor("idx", [P, NCOL * 8], i16, kind="ExternalInput")
    doff_t = nc.dram_tensor("dstoff", [P, NMM], i16, kind="ExternalInput")
    dinvp_t = nc.dram_tensor("dinvp", [P, J], f32, kind="ExternalInput")
    dinvcol_t = nc.dram_tensor("dinvcol", [HID, DPC], f32, kind="ExternalInput")
    x_t = nc.dram_tensor("xsh", [DPC, 3], f32, kind="ExternalInput")
    win_t = nc.dram_tensor("w_in", [3, HID], f32, kind="ExternalInput")
    bin_t = nc.dram_tensor("b_in", [HID, 1], f32, kind="ExternalInput")
    whid_t = nc.dram_tensor("w_hid", [max(NHID, 1), HID, HID], bf16, kind="ExternalInput")
    bhid_t = nc.dram_tensor("b_hid", [max(NHID, 1), HID, 1], f32, kind="ExternalInput")
    wout_t = nc.dram_tensor("w_out", [HID, 6], bf16, kind="ExternalInput")
    bout_t = nc.dram_tensor("b_out", [6, 1], f32, kind="ExternalInput")
    y_t = nc.dram_tensor("y", [DPC, 6], f32, kind="ExternalOutput")

    tables = [
        nc.dram_tensor(f"table{i}", [NTOT, FW], bf16, addr_space="Shared")
        for i in range(2)
    ]
    hsl = [nc.dram_tensor(f"hslice{i}", [DPC, FW], bf16) for i in range(2)]
    rg = [list(range(NCORES))]

    cmax = max(gr["cols"][1] - gr["cols"][0] for gr in groups)
    bmax = max(c1 - c0 for gr in groups for (c0, c1) in gr["bank_cols"])
    nmm_max = max(gr["mm1"] - gr["mm0"] for gr in groups)
    col_bank = np.zeros(NCOL, np.int64)
    for gr in groups:
        for b, (c0, c1) in enumerate(gr["bank_cols"]):
            col_bank[c0:c1] = b

    with tile.TileContext(nc, num_cores=NCORES) as tc:
        with (
            tc.tile_pool(name="const", bufs=1) as cpool,
            tc.tile_pool(name="mp", bufs=6) as mpool,
            tc.tile_pool(name="sp", bufs=3) as spool,
            tc.tile_pool(name="ip", bufs=2) as ipool,
            tc.tile_pool(name="atp", bufs=2) as atpool,
            tc.tile_pool(name="dvp", bufs=2) as dvpool,
            tc.tile_pool(name="rhp", bufs=2) as rhpool,
            tc.tile_pool(name="ps_sc", bufs=2, space="PSUM") as ps_sc,
            tc.tile_pool(name="ps_tr", bufs=2, space="PSUM") as ps_tr,
            tc.tile_pool(name="ps_tp", bufs=2, space="PSUM") as ps_tp,
        ):
            # ---- constants / resident tensors ----
            ident_f = cpool.tile([P, P], f32, tag="idf")
            make_identity(nc, ident_f[:])
            ident_b = cpool.tile([P, P], bf16, tag="idb")
            make_identity(nc, ident_b[:])
            iota64 = cpool.tile([P, W], i16, tag="iota")
            nc.gpsimd.iota(iota64[:], pattern=[[1, W]], base=0, channel_multiplier=0)
            w_in_sb = cpool.tile([3, HID], f32, tag="wi")
            nc.sync.dma_start(out=w_in_sb[:], in_=win_t[:])
            b_in_sb = cpool.tile([HID, 1], f32, tag="bi")
            nc.sync.dma_start(out=b_in_sb[:], in_=bin_t[:])
            whid_sb = cpool.tile([HID, max(NHID, 1) * HID], bf16, tag="wh")
            bhid_sb = cpool.tile([HID, max(NHID, 1)], f32, tag="bh")
            for l in range(max(NHID, 1)):
                nc.sync.dma_start(
                    out=whid_sb[:, l * HID : (l + 1) * HID], in_=whid_t[l, :, :]
                )
                nc.sync.dma_start(out=bhid_sb[:, l : l + 1], in_=bhid_t[l, :, :])
            wout_sb = cpool.tile([HID, 6], bf16, tag="wo")
            nc.sync.dma_start(out=wout_sb[:], in_=wout_t[:])
            bout_sb = cpool.tile([6, 1], f32, tag="bo")
            nc.sync.dma_start(out=bout_sb[:], in_=bout_t[:])

            doff_sb = cpool.tile([P, NMM], i16, tag="doff")
            nc.sync.dma_start(out=doff_sb[:], in_=doff_t[:])
            dinvp_sb = cpool.tile([P, J], f32, tag="dinvp")
            nc.sync.dma_start(out=dinvp_sb[:], in_=dinvp_t[:])

            # htr ping-pong: transposed own-slice features [P, J*64]
            htrA = cpool.tile([P, J * HID], bf16, tag="htrA")
            htrB = cpool.tile([P, J * HID], bf16, tag="htrB")
            htrs = [htrA, htrB]
            ytr = cpool.tile([P, J * 6], f32, tag="ytr")

            def publish_chunk(c, htr_src, r_next):
                """DMA htr chunk c into hsl + AllGather into tables[r_next%2]."""
                r0, r1 = int(CBASE[c]), int(CBASE[c] + CSZ[c])
                dst_h = hsl[r_next % 2]
                nc.sync.dma_start(
                    out=dst_h.ap()[r0:r1, :].rearrange("(j p) f -> p j f", p=P)[
                        :, :, 0:HID
                    ],
                    in_=htr_src[:, (r0 // P) * HID : (r1 // P) * HID].rearrange(
                        "p (j f) -> p j f", f=HID
                    ),
                )
                nc.gpsimd.collective_compute(
                    "AllGather", mybir.AluOpType.bypass, replica_groups=rg,
                    ins=[dst_h.ap()[r0:r1, :]],
                    outs=[
                        tables[r_next % 2].ap()[
                            NCORES * r0 : NCORES * r1, :
                        ]
                    ],
                )

            # ---- round -1: htr0 = (x @ W_in)^T slices; publish chunked ----
            htr0 = htrs[0]
            for j in range(J):
                xc = rhpool.tile([P, 3], f32, tag="xc")
                nc.sync.dma_start(out=xc[:], in_=x_t[j * P : (j + 1) * P, :])
                pxT = ps_tp.tile([3, P], f32, space="PSUM", tag="ptp")
                nc.tensor.transpose(out=pxT[:], in_=xc[:], identity=ident_f[:])
                xT = rhpool.tile([3, P], f32, tag="xT")
                nc.vector.tensor_copy(out=xT[:], in_=pxT[:])
                pt0 = ps_tr.tile([P, HID], f32, space="PSUM", tag="ptr")
                nc.tensor.matmul(
                    out=pt0[:], lhsT=xT[:], rhs=w_in_sb[:], start=True, stop=True
                )
                nc.scalar.activation(
                    out=htr0[:, j * HID : (j + 1) * HID], in_=pt0[:], func=AFT.Copy,
                    scale=dinvp_sb[:, j : j + 1],
                )
                if (j + 1) * P in [int(x) for x in CBASE[1:]] + [DPC]:
                    c = int(np.searchsorted(CBASE, j * P, side="right")) - 1
                    publish_chunk(c, htr0, 0)

            # ---- rounds ----
            qn = 0
            for r in range(NROUNDS):
                table = tables[r % 2]
                htr_prev = htrs[r % 2]
                htr_next = htrs[(r + 1) % 2]
                for gi, gr in enumerate(groups):
                    b0, b1 = gr["cols"]
                    ncolg = gr["ncols_psum"]
                    idx_sb = ipool.tile([P, cmax * 8], i16, tag="ix")
                    nc.sync.dma_start(
                        out=idx_sb[:, 0 : (b1 - b0) * 8],
                        in_=idx_t[:, b0 * 8 : b1 * 8],
                    )
                    # on-chip S for this group's matmuls
                    m0, m1 = gr["mm0"], gr["mm1"]
                    nmm_g = m1 - m0
                    s_sb = spool.tile([P, nmm_max, W], bf16, tag="s")
                    nc.vector.tensor_tensor(
                        out=s_sb[:, 0:nmm_g, :],
                        in0=iota64[:].unsqueeze(1).broadcast_to([P, nmm_g, W]),
                        in1=doff_sb[:, m0:m1].unsqueeze(2).broadcast_to([P, nmm_g, W]),
                        op=mybir.AluOpType.is_equal,
                    )
                    mtiles = []
                    for b in range(NB_):
                        c0, c1 = gr["bank_cols"][b]
                        if c1 == c0:
                            mtiles.append(None)
                            continue
                        mt = mpool.tile([P, bmax, FW], bf16, tag="m")
                        nidx = (c1 - c0) * P
                        nc.gpsimd.dma_gather(
                            out_ap=mt[:, 0 : c1 - c0, :],
                            in_ap=table[b * BANK : min((b + 1) * BANK, NTOT), :],
                            idxs_ap=idx_sb[:, (c0 - b0) * 8 : (c1 - b0) * 8],
                            num_idxs=nidx,
                            num_idxs_reg=nidx,
                            elem_size=FW,
                            single_packet=False,
                            queue_num=(gi + b) % 4,
                        )
                        mtiles.append((mt, c0))
                    psum = ps_sc.tile([HID, 512], f32, space="PSUM", tag="psc")
                    # self-loop diag matmuls zero-init the psum cols
                    jlist = list(range(4 * gi, min(4 * gi + 4, J)))
                    for jj, j in enumerate(jlist):
                        nc.tensor.matmul(
                            out=psum[:, jj * P : (jj + 1) * P],
                            lhsT=htr_prev[:, j * HID : (j + 1) * HID],
                            rhs=ident_b[:],
                            start=True, stop=False, skip_group_check=True,
                        )
                    for m in range(m0, m1):
                        c, w = int(mm_col[m]), int(mm_w[m])
                        wl = w - WPG * gi
                        bk = col_bank[c]
                        mt, cb = mtiles[bk]
                        nc.tensor.matmul(
                            out=psum[:, wl * W : (wl + 1) * W],
                            lhsT=mt[:, c - cb, 0:HID],
                            rhs=s_sb[:, m - m0, :],
                            start=False, stop=bool(mm_stop[m]),
                            skip_group_check=True,
                        )

                    # ---- evict with per-column dinv[dst] scale ----
                    dvc = dvpool.tile([HID, 512], f32, tag="dvc")
                    nc.sync.dma_start(
                        out=dvc[:, 0:ncolg],
                        in_=dinvcol_t[:, gi * 512 : gi * 512 + ncolg],
                    )
                    at = atpool.tile([HID, 512], bf16, tag="at")
                    nc.vector.tensor_tensor(
                        out=at[:, 0:ncolg], in0=psum[:, 0:ncolg],
                        in1=dvc[:, 0:ncolg], op=mybir.AluOpType.mult,
                    )
                    # ---- transform this group's 512-dst chunk ----
                    if r == 0:
                        hc = rhpool.tile([HID, 512], bf16, tag="hc")
                        nc.scalar.activation(
                            out=hc[:, 0:ncolg], in_=at[:, 0:ncolg],
                            func=AFT.Relu, bias=b_in_sb[:],
                        )
                    elif r < NROUNDS - 1:
                        pt = ps_tr.tile([HID, 512], f32, space="PSUM", tag="ptr")
                        nc.tensor.matmul(
                            out=pt[:, 0:ncolg],
                            lhsT=whid_sb[:, (r - 1) * HID : r * HID],
                            rhs=at[:, 0:ncolg], start=True, stop=True,
                        )
                        hc = rhpool.tile([HID, 512], bf16, tag="hc")
                        nc.scalar.activation(
                            out=hc[:, 0:ncolg], in_=pt[:, 0:ncolg], func=AFT.Relu,
                            bias=bhid_sb[:, r - 1 : r],
                        )
                    else:
                        pt = ps_tr.tile([6, 512], f32, space="PSUM", tag="pt6")
                        nc.tensor.matmul(
                            out=pt[:, 0:ncolg], lhsT=wout_sb[:],
                            rhs=at[:, 0:ncolg], start=True, stop=True,
                        )
                        yc = rhpool.tile([6, 512], f32, tag="yc")
                        nc.scalar.activation(
                            out=yc[:, 0:ncolg], in_=pt[:, 0:ncolg],
                            func=AFT.Sigmoid, bias=bout_sb[:],
                        )
                        for jj, j in enumerate(jlist):
                            ptp6 = ps_tp.tile([P, 6], f32, space="PSUM", tag="ptp")
                            nc.tensor.transpose(
                                out=ptp6[:], in_=yc[:, jj * P : (jj + 1) * P],
                                identity=ident_f[0:6, 0:6],
                            )
                            nc.vector.tensor_copy(
                                out=ytr[:, j * 6 : (j + 1) * 6], in_=ptp6[:]
                            )
                        continue

                    # transpose into htr_next
                    for jj, j in enumerate(jlist):
                        ptp = ps_tp.tile([P, HID], bf16, space="PSUM", tag="ptp")
                        nc.tensor.transpose(
                            out=ptp[:], in_=hc[:, jj * P : (jj + 1) * P],
                            identity=ident_b[0:HID, 0:HID],
                        )
                        nc.scalar.activation(
                            out=htr_next[:, j * HID : (j + 1) * HID], in_=ptp[:],
                            func=AFT.Copy, scale=dinvp_sb[:, j : j + 1],
                        )
                    # publish chunk when its last group is done
                    if (gi + 1) in CHUNK_ENDS:
                        publish_chunk(CHUNK_ENDS.index(gi + 1), htr_next, r + 1)

                if r == NROUNDS - 1:
                    nc.sync.dma_start(
                        out=y_t.ap().rearrange("(j p) f -> p j f", p=P),
                        in_=ytr[:].rearrange("p (j f) -> p j f", f=6),
                    )

    nc.compile()
    return nc


# ----------------------------------------------------------------- assembly --
def make_in_maps(inputs, pre, cfg):
    N, NCORES, DPC = cfg["N"], cfg["NCORES"], cfg["DPC"]
    NHID = max(cfg["NROUNDS"] - 2, 0)
    x = np.asarray(inputs["x"], np.float32)
    xpad = np.zeros((NCORES * DPC, 3), np.float32)
    xpad[:N] = x
    w_in = np.asarray(inputs["W_in"], np.float32)
    b_in = np.asarray(inputs["b_in"], np.float32).reshape(HID, 1)
    w_hid = np.asarray(inputs["W_hid"], np.float32)[:NHID]
    b_hid = np.asarray(inputs["b_hid"], np.float32)[:NHID]
    if NHID == 0:
        w_hid = np.zeros((1, HID, HID), np.float32)
        b_hid = np.zeros((1, HID), np.float32)
    w_out = np.asarray(inputs["W_out"], np.float32)
    b_out = np.asarray(inputs["b_out"], np.float32).reshape(6, 1)

    idxw = []
    for k in range(NCORES):
        a = pre["idx"][k]               # [P, NCOL] (lane, col)
        flat = a.T.reshape(-1)          # pos = col*128 + lane
        w16 = flat.reshape(-1, 16).T    # [16, NSLOT/16]
        idxw.append(np.ascontiguousarray(np.tile(w16, (8, 1))))

    in_maps = []
    for k in range(NCORES):
        in_maps.append(
            {
                "idx": idxw[k],
                "dstoff": np.ascontiguousarray(pre["dstoff"][k]),
                "dinvp": np.ascontiguousarray(pre["dinvp"][k]),
                "dinvcol": np.ascontiguousarray(pre["dinvcol"][k]),
                "xsh": np.ascontiguousarray(xpad[k * DPC : (k + 1) * DPC]),
                "w_in": w_in,
                "b_in": b_in,
                "w_hid": w_hid.astype(ml_dtypes.bfloat16),
                "b_hid": np.ascontiguousarray(b_hid.reshape(-1, HID, 1)).astype(np.float32),
                "w_out": w_out.astype(ml_dtypes.bfloat16),
                "b_out": b_out,
            }
        )
    return in_maps


def run(inputs, cfg=None, **spmd_kwargs):
    cfg = _cfg_derived(dict(cfg or REAL_CFG))
    edge_index = np.asarray(inputs["edge_index"])
    pre = preprocess(edge_index, cfg)
    nc = build_program(cfg, pre["sched"])
    in_maps = make_in_maps(inputs, pre, cfg)
    res = run_bass_kernel_spmd(
        nc, in_maps, core_ids=list(range(cfg["NCORES"])), **spmd_kwargs
    )
    y = np.concatenate([res.results[k]["y"] for k in range(cfg["NCORES"])])
    return y[: cfg["N"]].astype(np.float32), res


def kernel(**inputs):
    y, _ = run(inputs)
    return y
